# revision 2
# baseline (speedup 1.0000x reference)
"""Multi-head attention (B=2, L=S=2048, D=1024, H=16) on 8 Trainium2 cores.

Sharding: core c -> batch b = c // 4, head group g = c % 4 (4 heads per core).
W_Q/K/V column-sharded (256 cols per core), W_O row-sharded (256 rows per core);
the 4 partial outputs per batch are summed on the host (plus bias terms).

Per-core pipeline (all big tensors transposed so no on-device transposes needed):
  phase 1: QT = 0.125*(x Wq + bq)^T, KT = (x Wk + bk)^T  (feature-major [256, L])
           V  = x Wv (seq-major [S, 256], bias folded out: rows of softmax sum
           to 1, so A(V+1 bv^T) Wo = A V Wo + bv Wo -> host adds bv @ Wo + bo)
  phase 2: per (l-tile 512, s-tile 128): S^T = KT^T QT (row-packed pairs of
           heads, K=64), E = exp(S^T) * maskT (exp on ACT from PSUM, 0/1 mask
           multiply on DVE), AV^T += V^T E (col-packed 64-wide head pairs),
           rowsums += ones^T E (col-packed, yields dup-64 broadcast layout),
           then outT = AV^T * reciprocal(rowsums)
  phase 3: out_partial = outT^T Wo_rows (K=128 accumulation over pairs)

All matmul operands fp16 (1 cyc/row on PE, col-tiling legal); PSUM fp32.
"""
from contextlib import ExitStack

import numpy as np

import concourse.bass as bass
import concourse.mybir as mybir
import concourse.tile as tile
from concourse import bacc
from concourse.bass_utils import run_bass_kernel_spmd

F16 = mybir.dt.float16
F32 = mybir.dt.float32

D = 1024          # d_model
H = 16            # heads
DK = 64           # head dim
B, L = 2, 2048
S = L
NCORES = 8
HPC = 4           # heads per core
FPC = HPC * DK    # features per core = 256
KD = D // 128     # 8 contraction subtiles for projections
LT, LTW = 4, 512  # l tiles
ST, STW = 16, 128  # s tiles
Ident = mybir.ActivationFunctionType.Identity
Exp = mybir.ActivationFunctionType.Exp

_CACHED_NC = None


def _build():
    nc = bacc.Bacc("TRN2", target_bir_lowering=False, debug=False,
                   num_devices=NCORES)
    xT = nc.declare_dram_parameter("xT", [128, KD, L], F16, isOutput=False)
    wq = nc.declare_dram_parameter("wq", [128, KD, FPC], F16, isOutput=False)
    wk = nc.declare_dram_parameter("wk", [128, KD, FPC], F16, isOutput=False)
    wv = nc.declare_dram_parameter("wv", [128, KD, FPC], F16, isOutput=False)
    wo = nc.declare_dram_parameter("wo", [128, 2, D], F16, isOutput=False)
    bq = nc.declare_dram_parameter("bq", [128, 2], F32, isOutput=False)
    bk = nc.declare_dram_parameter("bk", [128, 2], F32, isOutput=False)
    maskT = nc.declare_dram_parameter("maskT", [ST, LT, 128, LTW], F16,
                                      isOutput=False)
    ones = nc.declare_dram_parameter("ones", [128, DK], F16, isOutput=False)
    out = nc.declare_dram_parameter("out", [128, ST, D], F32, isOutput=True)

    with tile.TileContext(nc) as tc, ExitStack() as ctx:
        wpool = ctx.enter_context(tc.tile_pool(name="wpool", bufs=1))
        qkv = ctx.enter_context(tc.tile_pool(name="qkv", bufs=1))

        wq_sb = wpool.tile([128, KD, FPC], F16)
        wk_sb = wpool.tile([128, KD, FPC], F16)
        wv_sb = wpool.tile([128, KD, FPC], F16)
        wo_sb = wpool.tile([128, 2, D], F16)
        bq_sb = wpool.tile([128, 2], F32)
        bk_sb = wpool.tile([128, 2], F32)
        ones_sb = wpool.tile([128, DK], F16)
        nc.sync.dma_start(out=wq_sb[:], in_=wq[:])
        nc.sync.dma_start(out=wk_sb[:], in_=wk[:])
        nc.sync.dma_start(out=wv_sb[:], in_=wv[:])
        nc.sync.dma_start(out=wo_sb[:], in_=wo[:])
        nc.sync.dma_start(out=bq_sb[:], in_=bq[:])
        nc.sync.dma_start(out=bk_sb[:], in_=bk[:])
        nc.sync.dma_start(out=ones_sb[:], in_=ones[:])

        QT = qkv.tile([128, 2, L], F16)   # [feat(2x128), l]: Q^T * 0.125
        KT = qkv.tile([128, 2, L], F16)
        V = qkv.tile([128, ST, FPC], F16)  # [s, st, feat]
        outT = qkv.tile([128, 2, L], F16)  # [d(2x64 per pair), pair, l]

        # ---------------- phase 1: projections ----------------
        with tc.tile_pool(name="xt", bufs=1) as xpool, \
             tc.tile_pool(name="pp1", bufs=4, space="PSUM") as pp1:
            xt = xpool.tile([128, KD, L], F16)
            nc.sync.dma_start(out=xt[:], in_=xT[:])

            for ft in range(2):
                for lt in range(LT):
                    lsl = slice(lt * LTW, (lt + 1) * LTW)
                    fsl = slice(ft * 128, (ft + 1) * 128)
                    psq = pp1.tile([128, LTW], F32, tag="ps1")
                    for kd in range(KD):
                        nc.tensor.matmul(psq[:], wq_sb[:, kd, fsl],
                                         xt[:, kd, lsl],
                                         start=(kd == 0), stop=(kd == KD - 1))
                    nc.scalar.activation(QT[:, ft, lsl], psq[:], Ident,
                                         bias=bq_sb[:, ft:ft + 1], scale=0.125)
                    psk = pp1.tile([128, LTW], F32, tag="ps1")
                    for kd in range(KD):
                        nc.tensor.matmul(psk[:], wk_sb[:, kd, fsl],
                                         xt[:, kd, lsl],
                                         start=(kd == 0), stop=(kd == KD - 1))
                    nc.scalar.activation(KT[:, ft, lsl], psk[:], Ident,
                                         bias=bk_sb[:, ft:ft + 1])
            for st in range(ST):
                ssl = slice(st * STW, (st + 1) * STW)
                psv = pp1.tile([128, FPC], F32, tag="psv")
                for kd in range(KD):
                    nc.tensor.matmul(psv[:], xt[:, kd, ssl], wv_sb[:, kd, :],
                                     start=(kd == 0), stop=(kd == KD - 1))
                nc.vector.tensor_copy(V[:, st, :], psv[:])

        # ---------------- phase 2: attention ----------------
        with tc.tile_pool(name="mpool", bufs=3) as mpool, \
             tc.tile_pool(name="epool", bufs=4) as epool, \
             tc.tile_pool(name="rbpool", bufs=2) as rbpool, \
             tc.tile_pool(name="scp", bufs=2, space="PSUM") as scp, \
             tc.tile_pool(name="avp", bufs=1, space="PSUM") as avp, \
             tc.tile_pool(name="rsp", bufs=1, space="PSUM") as rsp:
            for lt in range(LT):
                lsl = slice(lt * LTW, (lt + 1) * LTW)
                av0 = avp.tile([128, LTW], F32, tag="av0")
                av1 = avp.tile([128, LTW], F32, tag="av1")
                rs0 = rsp.tile([128, LTW], F32, tag="rs0")
                rs1 = rsp.tile([128, LTW], F32, tag="rs1")
                avs, rss = (av0, av1), (rs0, rs1)
                for st in range(ST):
                    ssl = slice(st * STW, (st + 1) * STW)
                    mk = mpool.tile([128, LTW], F16)
                    nc.sync.dma_start(out=mk[:], in_=maskT[st, lt])
                    for pair in range(2):
                        sc = scp.tile([128, 2, LTW], F32, tag="sc")
                        for i in range(2):
                            nc.tensor.matmul(
                                sc[:, i, :],
                                KT[64 * i:64 * (i + 1), pair, ssl],
                                QT[64 * i:64 * (i + 1), pair, lsl],
                                start=True, stop=True)
                        E = epool.tile([128, 2, LTW], F16)
                        nc.scalar.activation(E[:], sc[:], Exp)
                        nc.vector.tensor_mul(
                            E[:], E[:],
                            mk[:, None, :].to_broadcast((128, 2, LTW)))
                        for i in range(2):
                            h = 2 * pair + i
                            nc.tensor.matmul(
                                avs[pair][64 * i:64 * (i + 1), :],
                                V[:, st, DK * h:DK * (h + 1)], E[:, i, :],
                                start=(st == 0), stop=(st == ST - 1),
                                tile_position=(0, 64 * i))
                            nc.tensor.matmul(
                                rss[pair][64 * i:64 * (i + 1), :],
                                ones_sb[:], E[:, i, :],
                                start=(st == 0), stop=(st == ST - 1),
                                tile_position=(0, 64 * i))
                for pair in range(2):
                    rb = rbpool.tile([128, LTW], F32)
                    nc.vector.reciprocal(rb[:], rss[pair][:])
                    nc.vector.tensor_mul(outT[:, pair, lsl], avs[pair][:],
                                         rb[:])

        # ---------------- phase 3: output projection ----------------
        with tc.tile_pool(name="opool", bufs=3) as opool, \
             tc.tile_pool(name="pp3", bufs=4, space="PSUM") as pp3:
            for lt8 in range(ST):
                csl = slice(lt8 * 128, (lt8 + 1) * 128)
                for nf in range(2):
                    nsl = slice(nf * 512, (nf + 1) * 512)
                    ps = pp3.tile([128, 512], F32, tag="ps3")
                    for pair in range(2):
                        nc.tensor.matmul(ps[:], outT[:, pair, csl],
                                         wo_sb[:, pair, nsl],
                                         start=(pair == 0), stop=(pair == 1))
                    ob = opool.tile([128, 512], F32)
                    nc.vector.tensor_copy(ob[:], ps[:])
                    nc.gpsimd.dma_start(out=out[:, lt8, nsl], in_=ob[:])

    nc.compile()
    return nc


def _get_nc():
    global _CACHED_NC
    if _CACHED_NC is None:
        _CACHED_NC = _build()
    return _CACHED_NC


def _prep_core_inputs(c, x, mask, Wq, bq, Wk, bk, Wv, Wo):
    b, g = divmod(c, 4)
    cs = slice(g * FPC, (g + 1) * FPC)

    xT = np.ascontiguousarray(
        x[b].T.reshape(KD, 128, L).transpose(1, 0, 2)).astype(np.float16)
    wq_c = np.ascontiguousarray(
        Wq[:, cs].reshape(KD, 128, FPC).transpose(1, 0, 2)).astype(np.float16)
    wk_c = np.ascontiguousarray(
        Wk[:, cs].reshape(KD, 128, FPC).transpose(1, 0, 2)).astype(np.float16)
    wv_c = np.ascontiguousarray(
        Wv[:, cs].reshape(KD, 128, FPC).transpose(1, 0, 2)).astype(np.float16)
    wo_c = np.ascontiguousarray(
        Wo[cs, :].reshape(2, 128, D).transpose(1, 0, 2)).astype(np.float16)
    bq_c = np.ascontiguousarray(
        (bq[cs] * 0.125).reshape(2, 128).T).astype(np.float32)
    bk_c = np.ascontiguousarray(bk[cs].reshape(2, 128).T).astype(np.float32)
    mT = mask[b].astype(np.float16).T  # [S, L]
    maskT = np.ascontiguousarray(
        mT.reshape(ST, 128, LT, LTW).transpose(0, 2, 1, 3))
    ones = np.ones((128, DK), np.float16)
    return {"xT": xT, "wq": wq_c, "wk": wk_c, "wv": wv_c, "wo": wo_c,
            "bq": bq_c, "bk": bk_c, "maskT": maskT, "ones": ones}


def kernel(x, mask, Wq, bq, Wk, bk, Wv, bv, Wo, bo):
    x = np.asarray(x, np.float32)
    mask = np.asarray(mask)
    Wq, bq = np.asarray(Wq, np.float32), np.asarray(bq, np.float32)
    Wk, bk = np.asarray(Wk, np.float32), np.asarray(bk, np.float32)
    Wv, bv = np.asarray(Wv, np.float32), np.asarray(bv, np.float32)
    Wo, bo = np.asarray(Wo, np.float32), np.asarray(bo, np.float32)

    nc = _get_nc()
    in_maps = [_prep_core_inputs(c, x, mask, Wq, bq, Wk, bk, Wv, Wo)
               for c in range(NCORES)]
    res = run_bass_kernel_spmd(nc, in_maps, list(range(NCORES)))

    const_vec = (bv @ Wo + bo).astype(np.float32)  # A rows sum to 1
    outs = []
    for b in range(B):
        acc = np.zeros((L, D), np.float32)
        for g in range(4):
            part = res.results[4 * b + g]["out"]  # [128, 16, 1024]
            acc += part.transpose(1, 0, 2).reshape(L, D)
        acc += const_vec
        outs.append(acc)
    return np.stack(outs)
